# revision 8
# baseline (speedup 1.0000x reference)
"""Trainium2 Bass kernel for nn_Diffusion_Model (ragged_sequence).

Pure data-parallel: batch B=4096 sharded as 512 per NeuronCore across 8 cores.

Per-core design (token layout: partitions = node n, free = per-token values):
  - x is host-padded with a ones-row -> per-b lhsT [65,128] so one PE matmul
    computes both h = x@W11+b11 (cols 0..63) and the raw transpose
    xT = x.T (cols 64..127, via an identity block in the rhs).
  - LayerNorm stats via DVE bn_stats (even/odd chunk merge), relu via ACT
    with per-partition -mu bias, W12 dot via broadcast multiply + segmented
    reduce. rstd is factored out of the relu (g=1, beta=0 fast path; general
    gamma/beta handled with two extra passes).
  - branch 2 ("down" channel) collapses over n: one [65,512] staged matrix,
    ARCH-A style stats via ones-matmuls, broadcast back over partitions with
    a K=1 matmul.
  - Ragged geometric tail as a Horner scan along t (tensor_tensor_scan,
    a = q with per-segment reset columns), extraction of s[Ln-1] either as
    the last column (when the weights provably force Ln==64) or via
    tensor_mask_reduce per batch.
  - Final partition reduction with a ones-vector matmul.
"""
import sys
import numpy as np

sys.path.insert(0, "/opt/trn_rl_repo")

import concourse.bacc as bacc
import concourse.tile as tile
import concourse.mybir as mybir
from concourse.bass_utils import run_bass_kernel_spmd

dt = mybir.dt
Alu = mybir.AluOpType
Act = mybir.ActivationFunctionType
Ax = mybir.AxisListType

N_CORES = 8
B, T, N, H = 4096, 64, 128, 64
LN_EPS = 1e-5


def _np32(a):
    return np.ascontiguousarray(np.asarray(a, dtype=np.float32))


class _Built:
    pass


def _build(weights, BPC, ln_affine, general_tail):
    """Build the per-core Bass program. weights: dict of host-folded arrays."""
    NG = BPC // 8  # groups of 8 batches
    nc = bacc.Bacc("TRN2", target_bir_lowering=False, debug=False,
                   num_devices=N_CORES)

    x_in = nc.dram_tensor("x", [BPC, T + 1, N + 1], dt.float32,
                          kind="ExternalInput").ap()
    y_out = nc.dram_tensor("y", [BPC, 1], dt.float32,
                           kind="ExternalOutput").ap()

    # ---- inline constants ----
    W11, b11 = weights["W11"], weights["b11"]     # [64,64],[64]
    W21, b21 = weights["W21"], weights["b21"]
    W12, b12 = weights["W12"], float(weights["b12"])   # [64], scalar
    W22, b22 = weights["W22"], float(weights["b22"])
    g11, be11 = weights["g11"], weights["be11"]
    g21, be21 = weights["g21"], weights["be21"]
    w30, w31, b3 = (float(weights["w30"]), float(weights["w31"]),
                    float(weights["b3"]))
    alpha = float(weights["alpha"])

    RW = np.zeros((T + 1, 2 * H), np.float32)
    RW[:T, :H] = W11
    RW[T, :H] = b11
    RW[:T, H:] = np.eye(T, dtype=np.float32)
    RW2 = np.zeros((T + 1, H), np.float32)
    RW2[:T, :] = W21
    RW2[T, :] = b21

    w12bc = np.broadcast_to(W12[None, :], (128, H)).copy()
    g11bc = np.broadcast_to(g11[None, :], (128, H)).copy()
    be11bc = np.broadcast_to(be11[None, :], (128, H)).copy()
    t0m = np.ones((128, 512), np.float32)
    t0m[:, ::T] = 0.0
    ones_col = np.ones((128, 1), np.float32)
    ones64_col = np.ones((T, 1), np.float32)
    noneg64_row = np.full((1, H), -1.0 / H, np.float32)
    w22_col = W22.reshape(T, 1).astype(np.float32)

    c_RW = nc.inline_tensor(RW, "c_RW")
    c_RW2 = nc.inline_tensor(RW2, "c_RW2")
    c_w12bc = nc.inline_tensor(w12bc, "c_w12bc")
    c_t0m = nc.inline_tensor(t0m, "c_t0m")
    c_ones = nc.inline_tensor(ones_col, "c_ones")
    c_ones64 = nc.inline_tensor(ones64_col, "c_ones64")
    c_negmean = nc.inline_tensor(noneg64_row, "c_negmean")
    c_w22 = nc.inline_tensor(w22_col, "c_w22")
    if ln_affine:
        c_g11bc = nc.inline_tensor(g11bc, "c_g11bc")
        c_be11bc = nc.inline_tensor(be11bc, "c_be11bc")

    def _chunks(total, step=512):
        return [slice(i, min(i + step, total)) for i in range(0, total, step)]

    with tile.TileContext(nc) as tc:
        with tc.tile_pool(name="const", bufs=1) as cpool:
            RW_sb = cpool.tile([T + 1, 2 * H], dt.float32)
            nc.sync.dma_start(RW_sb[:], c_RW.ap())
            RW2_sb = cpool.tile([T + 1, H], dt.float32)
            nc.sync.dma_start(RW2_sb[:], c_RW2.ap())
            w12_sb = cpool.tile([128, H], dt.float32)
            nc.sync.dma_start(w12_sb[:], c_w12bc.ap())
            t0m_sb = cpool.tile([128, 512], dt.float32)
            nc.sync.dma_start(t0m_sb[:], c_t0m.ap())
            ones_sb = cpool.tile([128, 1], dt.float32)
            nc.sync.dma_start(ones_sb[:], c_ones.ap())
            ones64_sb = cpool.tile([T, 1], dt.float32)
            nc.sync.dma_start(ones64_sb[:], c_ones64.ap())
            negmean_sb = cpool.tile([1, H], dt.float32)
            nc.sync.dma_start(negmean_sb[:], c_negmean.ap())
            w22_sb = cpool.tile([T, 1], dt.float32)
            nc.sync.dma_start(w22_sb[:], c_w22.ap())
            if ln_affine:
                g11_sb = cpool.tile([128, H], dt.float32)
                nc.sync.dma_start(g11_sb[:], c_g11bc.ap())
                be11_sb = cpool.tile([128, H], dt.float32)
                nc.sync.dma_start(be11_sb[:], c_be11bc.ap())

            eps_sb = cpool.tile([128, 1], dt.float32)
            nc.vector.memset(eps_sb[:], LN_EPS)
            b12_sb = cpool.tile([128, 1], dt.float32)
            nc.vector.memset(b12_sb[:], b12)
            b22_sb = cpool.tile([128, 1], dt.float32)
            nc.vector.memset(b22_sb[:], b22)

            xdbc_sb = cpool.tile([128, BPC], dt.float32)   # xd broadcast
            acc_sb = cpool.tile([128, BPC], dt.float32)    # F * s* per token

            # ================= branch 2 (down channel), once =================
            with tc.tile_pool(name="b2", bufs=1) as b2, \
                 tc.tile_pool(name="b2ps", bufs=1, space="PSUM") as b2ps:
                onesrow = b2.tile([1, 128], dt.float32)
                nc.vector.memset(onesrow[:], 1.0)
                dstage = b2.tile([T + 1, BPC], dt.float32)
                # strided gather of the down column (+ ones row from padding)
                nc.sync.dma_start(
                    dstage[:],
                    x_in.rearrange("b t n -> t b n")[:, :, N:N + 1],
                )
                ps2 = b2ps.tile([H, BPC], dt.float32)
                for sl in _chunks(BPC):
                    nc.tensor.matmul(ps2[:, sl], RW2_sb[:], dstage[:, sl],
                                     start=True, stop=True)
                h2sb = b2.tile([H, BPC], dt.float32)
                nc.scalar.copy(h2sb[:], ps2[:])
                sq2 = b2.tile([H, BPC], dt.float32)
                nc.scalar.square(sq2[:], ps2[:])
                psS1 = b2ps.tile([1, BPC], dt.float32)
                psS2 = b2ps.tile([1, BPC], dt.float32)
                for sl in _chunks(BPC):
                    nc.tensor.matmul(psS1[0:1, sl], ones64_sb[:], h2sb[:, sl],
                                     start=True, stop=True)
                    nc.tensor.matmul(psS2[0:1, sl], ones64_sb[:], sq2[:, sl],
                                     start=True, stop=True)
                S1row = b2.tile([1, BPC], dt.float32)
                nc.vector.tensor_copy(S1row[:], psS1[0:1, :])
                # centered h2 in a fresh bank: redo mm then accumulate -mu
                ps2c = b2ps.tile([H, BPC], dt.float32)
                for sl in _chunks(BPC):
                    nc.tensor.matmul(ps2c[:, sl], RW2_sb[:], dstage[:, sl],
                                     start=True, stop=False)
                for sl in _chunks(BPC):
                    nc.tensor.matmul(ps2c[:, sl], negmean_sb[:], S1row[:, sl],
                                     start=False, stop=True)
                rl2 = b2.tile([H, BPC], dt.float32)
                if ln_affine:
                    # general gamma/beta for branch 2: nh*g+be then relu
                    mu2r = b2.tile([1, BPC], dt.float32)
                    nc.vector.tensor_scalar(mu2r[:], S1row[:], 1.0 / H, None,
                                            Alu.mult)
                    var2r = b2.tile([1, BPC], dt.float32)
                    nc.vector.tensor_scalar(var2r[:], psS2[0:1, :], 1.0 / H,
                                            None, Alu.mult)
                    mu2sq = b2.tile([1, BPC], dt.float32)
                    nc.vector.tensor_tensor(mu2sq[:], mu2r[:], mu2r[:],
                                            Alu.mult)
                    nc.vector.tensor_tensor(var2r[:], var2r[:], mu2sq[:],
                                            Alu.subtract)
                    sd2 = b2.tile([1, BPC], dt.float32)
                    nc.scalar.activation(sd2[:], var2r[:], Act.Sqrt,
                                         bias=eps_sb[0:1, :])
                    rstd2r = b2.tile([1, BPC], dt.float32)
                    nc.vector.reciprocal(rstd2r[:], sd2[:])
                    # nh = hc * rstd (bcast via K=1 matmul) ... then *g+be
                    psb = b2ps.tile([H, BPC], dt.float32)
                    for sl in _chunks(BPC):
                        nc.tensor.matmul(psb[:, sl], onesrow[:, 0:H],
                                         rstd2r[:, sl], start=True, stop=True)
                    rsb = b2.tile([H, BPC], dt.float32)
                    nc.vector.tensor_copy(rsb[:], psb[:])
                    nh2 = b2.tile([H, BPC], dt.float32)
                    nc.vector.tensor_tensor(nh2[:], ps2c[:], rsb[:], Alu.mult)
                    g2 = np.broadcast_to(g21[:, None], (H, 1)).copy()
                    be2 = np.broadcast_to(be21[:, None], (H, 1)).copy()
                    c_g2 = nc.inline_tensor(g2.astype(np.float32), "c_g2")
                    c_be2 = nc.inline_tensor(be2.astype(np.float32), "c_be2")
                    g2_sb = b2.tile([H, 1], dt.float32)
                    nc.sync.dma_start(g2_sb[:], c_g2.ap())
                    be2_sb = b2.tile([H, 1], dt.float32)
                    nc.sync.dma_start(be2_sb[:], c_be2.ap())
                    nc.vector.tensor_scalar(nh2[:], nh2[:], g2_sb[:],
                                            be2_sb[:], Alu.mult, Alu.add)
                    nc.scalar.activation(rl2[:], nh2[:], Act.Relu)
                else:
                    nc.scalar.activation(rl2[:], ps2c[:], Act.Relu)
                psD = b2ps.tile([1, BPC], dt.float32)
                for sl in _chunks(BPC):
                    nc.tensor.matmul(psD[0:1, sl], w22_sb[:], rl2[:, sl],
                                     start=True, stop=True)
                # xd = sigmoid(rstd2*dots2 + b22)  (fast path)
                #      sigmoid(dots2 + b22)        (affine path: rstd inside)
                xdrow = b2.tile([1, BPC], dt.float32)
                if ln_affine:
                    nc.scalar.activation(xdrow[:], psD[0:1, :], Act.Sigmoid,
                                         bias=b22_sb[0:1, :])
                else:
                    S2row = b2.tile([1, BPC], dt.float32)
                    nc.vector.tensor_scalar(S2row[:], psS2[0:1, :], 1.0 / H,
                                            None, Alu.mult)
                    mu2r = b2.tile([1, BPC], dt.float32)
                    nc.vector.tensor_scalar(mu2r[:], S1row[:], 1.0 / H, None,
                                            Alu.mult)
                    mu2sq = b2.tile([1, BPC], dt.float32)
                    nc.vector.tensor_tensor(mu2sq[:], mu2r[:], mu2r[:],
                                            Alu.mult)
                    var2 = b2.tile([1, BPC], dt.float32)
                    nc.vector.tensor_tensor(var2[:], S2row[:], mu2sq[:],
                                            Alu.subtract)
                    sd2 = b2.tile([1, BPC], dt.float32)
                    nc.scalar.activation(sd2[:], var2[:], Act.Sqrt,
                                         bias=eps_sb[0:1, :])
                    rstd2 = b2.tile([1, BPC], dt.float32)
                    nc.vector.reciprocal(rstd2[:], sd2[:])
                    xin2 = b2.tile([1, BPC], dt.float32)
                    nc.vector.tensor_tensor(xin2[:], psD[0:1, :], rstd2[:],
                                            Alu.mult)
                    nc.scalar.activation(xdrow[:], xin2[:], Act.Sigmoid,
                                         bias=b22_sb[0:1, :])
                # broadcast xd over partitions: K=1 ones matmul
                psX = b2ps.tile([128, BPC], dt.float32)
                for sl in _chunks(BPC):
                    nc.tensor.matmul(psX[:, sl], onesrow[:], xdrow[:, sl],
                                     start=True, stop=True)
                nc.vector.tensor_copy(xdbc_sb[:], psX[:])

            # ================= main loop over groups of 8 b =================
            with tc.tile_pool(name="xt", bufs=3) as xpool, \
                 tc.tile_pool(name="ps", bufs=3, space="PSUM") as pspool, \
                 tc.tile_pool(name="big", bufs=3) as bigp, \
                 tc.tile_pool(name="sm", bufs=4) as smp:
                for g in range(NG):
                    xt = xpool.tile([T + 1, 8 * (N + 1)], dt.float32)
                    nc.sync.dma_start(
                        xt[:],
                        x_in.rearrange("b t n -> t b n")[:, g * 8:(g + 1) * 8, :],
                    )
                    ps = pspool.tile([128, 1024], dt.float32)
                    for j in range(8):
                        lhs = xt[:, j * (N + 1): j * (N + 1) + N]
                        nc.tensor.matmul(ps[:, j * H:(j + 1) * H], lhs,
                                         RW_sb[:, 0:H], start=True, stop=True)
                        nc.tensor.matmul(ps[:, 512 + j * T:512 + (j + 1) * T],
                                         lhs, RW_sb[:, H:2 * H],
                                         start=True, stop=True)
                    hv = ps[:, 0:512].rearrange("p (g t) -> p g t", g=8)
                    sqv = bigp.tile([128, 512], dt.float32, tag="sqv")
                    nc.scalar.square(sqv[:], ps[:, 0:512])
                    mus = smp.tile([128, 8], dt.float32, tag="mus")
                    nc.vector.tensor_reduce(mus[:], hv, Ax.X, Alu.add)
                    sqs = smp.tile([128, 8], dt.float32, tag="sqs")
                    nc.vector.tensor_reduce(
                        sqs[:], sqv[:].rearrange("p (g t) -> p g t", g=8),
                        Ax.X, Alu.add)
                    negmu = smp.tile([128, 8], dt.float32, tag="negmu")
                    nc.vector.tensor_scalar(negmu[:], mus[:], -1.0 / H, None,
                                            Alu.mult)
                    # var = sqs/64 - mu^2
                    mu2 = smp.tile([128, 8], dt.float32, tag="mu2")
                    nc.vector.tensor_tensor(mu2[:], negmu[:], negmu[:],
                                            Alu.mult)
                    var = smp.tile([128, 8], dt.float32, tag="var")
                    nc.vector.tensor_scalar(var[:], sqs[:], 1.0 / H, None,
                                            Alu.mult)
                    nc.vector.tensor_tensor(var[:], var[:], mu2[:],
                                            Alu.subtract)
                    sd = smp.tile([128, 8], dt.float32, tag="sd")
                    nc.scalar.activation(sd[:], var[:], Act.Sqrt, bias=eps_sb[:])
                    rstd = smp.tile([128, 8], dt.float32, tag="rstd")
                    nc.vector.reciprocal(rstd[:], sd[:])

                    rl = bigp.tile([128, 512], dt.float32, tag="rl")
                    for j in range(8):
                        nc.scalar.activation(
                            rl[:, j * H:(j + 1) * H], ps[:, j * H:(j + 1) * H],
                            Act.Relu, bias=negmu[:, j:j + 1], scale=1.0)
                    if ln_affine:
                        # nh = relu((h-mu))*rstd*g + be ... needs full form:
                        # recompute: nh = (h-mu)*rstd*g+be; relu after.
                        nh = bigp.tile([128, 512], dt.float32, tag="nh")
                        # (h-mu): redo via ts with per-seg scalar is not
                        # available; use rl trick only valid for g>0.
                        # General path: nh = sign-aware — implemented as
                        # ((h-mu)*rstd)*g + be with h-mu from ACT Identity.
                        hm = bigp.tile([128, 512], dt.float32, tag="hm")
                        for j in range(8):
                            nc.scalar.activation(
                                hm[:, j * H:(j + 1) * H],
                                ps[:, j * H:(j + 1) * H],
                                Act.Identity, bias=negmu[:, j:j + 1],
                                scale=1.0)
                        hm3 = hm[:].rearrange("p (g t) -> p g t", g=8)
                        rst_b = rstd[:].unsqueeze(2).broadcast_to([128, 8, H])
                        nc.vector.tensor_tensor(hm3, hm3, rst_b, Alu.mult)
                        g_b = g11_sb[:].unsqueeze(1).broadcast_to([128, 8, H])
                        be_b = be11_sb[:].unsqueeze(1).broadcast_to([128, 8, H])
                        nh3 = nh[:].rearrange("p (g t) -> p g t", g=8)
                        nc.vector.tensor_tensor(nh3, hm3, g_b, Alu.mult)
                        nc.vector.tensor_tensor(nh3, nh3, be_b, Alu.add)
                        nc.vector.tensor_scalar(rl[:], nh[:], 0.0, None,
                                                Alu.max)
                    dotp = bigp.tile([128, 512], dt.float32, tag="dotp")
                    w12v = w12_sb[:].unsqueeze(1).broadcast_to([128, 8, H])
                    rl3 = rl[:].rearrange("p (g t) -> p g t", g=8)
                    nc.vector.tensor_tensor(
                        dotp[:].rearrange("p (g t) -> p g t", g=8),
                        rl3, w12v, Alu.mult)
                    dots = smp.tile([128, 8], dt.float32, tag="dots")
                    nc.vector.tensor_reduce(
                        dots[:], dotp[:].rearrange("p (g t) -> p g t", g=8),
                        Ax.X, Alu.add)

                    xin = smp.tile([128, 8], dt.float32, tag="xin")
                    if ln_affine:
                        nc.vector.tensor_copy(xin[:], dots[:])
                    else:
                        nc.vector.tensor_tensor(xin[:], dots[:], rstd[:],
                                                Alu.mult)
                    xu = smp.tile([128, 8], dt.float32, tag="xu")
                    nc.scalar.activation(xu[:], xin[:], Act.Sigmoid, bias=b12_sb[:])

                    # v + 1e-5 = w30*xu + (w31*xd + b3 + 1e-5)
                    vx = smp.tile([128, 8], dt.float32, tag="vx")
                    nc.vector.tensor_scalar(vx[:],
                                            xdbc_sb[:, g * 8:(g + 1) * 8],
                                            w31, b3 + 1e-5, Alu.mult, Alu.add)
                    v1 = smp.tile([128, 8], dt.float32, tag="v1")
                    nc.vector.tensor_scalar(v1[:], xu[:], w30, None, Alu.mult)
                    nc.vector.tensor_tensor(v1[:], v1[:], vx[:], Alu.add)
                    rr = smp.tile([128, 8], dt.float32, tag="rr")
                    nc.vector.reciprocal(rr[:], v1[:])
                    fden = smp.tile([128, 8], dt.float32, tag="fden")
                    nc.vector.tensor_scalar(fden[:], rr[:], 50.0 * alpha, 1.0,
                                            Alu.mult, Alu.add)
                    F = smp.tile([128, 8], dt.float32, tag="F")
                    nc.vector.reciprocal(F[:], fden[:])
                    q = smp.tile([128, 8], dt.float32, tag="q")
                    nc.vector.tensor_scalar(q[:], F[:], -1.0, 1.0, Alu.mult,
                                            Alu.add)

                    if general_tail:
                        y5 = smp.tile([128, 8], dt.float32, tag="y5")
                        nc.vector.tensor_scalar(y5[:], rr[:], 5.0, 0.5,
                                                Alu.mult, Alu.add)
                        yi = smp.tile([128, 8], dt.int32, tag="yi")
                        nc.vector.tensor_copy(yi[:], y5[:])
                        yf = smp.tile([128, 8], dt.float32, tag="yf")
                        nc.vector.tensor_copy(yf[:], yi[:])
                        Tc = smp.tile([128, 8], dt.float32, tag="Tc")
                        nc.vector.tensor_scalar(Tc[:], yf[:], 0.0, 63.0,
                                                Alu.max, Alu.min)
                        mst = smp.tile([128, 8], dt.float32, tag="mst")
                        nc.vector.tensor_scalar(mst[:], Tc[:], -1.0, 63.0,
                                                Alu.mult, Alu.add)
                        men = smp.tile([128, 8], dt.float32, tag="men")
                        nc.vector.tensor_scalar(men[:], Tc[:], -1.0, 64.0,
                                                Alu.mult, Alu.add)

                    # a = q (0-step bcast) * t0mask; scan; extract; accumulate
                    a = bigp.tile([128, 512], dt.float32, tag="a")
                    qb = q[:].unsqueeze(2).broadcast_to([128, 8, T])
                    nc.vector.tensor_tensor(
                        a[:].rearrange("p (g t) -> p g t", g=8), qb,
                        t0m_sb[:].rearrange("p (g t) -> p g t", g=8), Alu.mult)
                    s = bigp.tile([128, 512], dt.float32, tag="s")
                    nc.vector.tensor_tensor_scan(s[:], a[:], ps[:, 512:1024],
                                                 0.0, Alu.mult, Alu.add)
                    accs = acc_sb[:, g * 8:(g + 1) * 8]
                    if general_tail:
                        sstar = smp.tile([128, 8], dt.float32, tag="sstar")
                        junk = bigp.tile([128, 64], dt.float32, tag="junk")
                        for j in range(8):
                            nc.vector.tensor_mask_reduce(
                                junk[:], s[:, j * T:(j + 1) * T],
                                mst[:, j:j + 1], men[:, j:j + 1], 1.0,
                                -3.0e38, Alu.max,
                                accum_out=sstar[:, j:j + 1])
                        nc.vector.tensor_tensor(accs, sstar[:], F[:], Alu.mult)
                    else:
                        slast = s[:].rearrange("p (g t) -> p g t", g=8)[:, :, T - 1]
                        nc.vector.tensor_tensor(accs, slast, F[:], Alu.mult)

                # ---- final: pred[b] = sum over partitions of acc ----
                with tc.tile_pool(name="fin", bufs=1) as fin, \
                     tc.tile_pool(name="finps", bufs=1, space="PSUM") as fps:
                    po = fps.tile([1, BPC], dt.float32)
                    for sl in _chunks(BPC):
                        nc.tensor.matmul(po[0:1, sl], ones_sb[:],
                                         acc_sb[:, sl], start=True, stop=True)
                    pred = fin.tile([1, BPC], dt.float32)
                    nc.vector.tensor_copy(pred[:], po[0:1, :])
                    nc.sync.dma_start(
                        y_out.rearrange("b one -> one b"), pred[:])

    nc.compile()
    built = _Built()
    built.nc = nc
    built.BPC = BPC
    return built


_CACHE = {}


def _get_built(weights, BPC, ln_affine, general_tail):
    key = (BPC, ln_affine, general_tail,
           hash(tuple(np.asarray(v).tobytes() for v in (
               weights["W11"].ravel()[:4], weights["b11"].ravel()[:4]))))
    # cache on full weight bytes to be safe
    full_key = (BPC, ln_affine, general_tail,
                b"".join(_np32(weights[k]).tobytes() for k in sorted(weights)))
    if full_key not in _CACHE:
        _CACHE[full_key] = _build(weights, BPC, ln_affine, general_tail)
    return _CACHE[full_key]


def _fold_weights(inputs):
    mean = float(np.asarray(inputs["x_mean"]))
    std = float(np.asarray(inputs["x_std"]))
    W11r = _np32(inputs["W11"])
    W21r = _np32(inputs["W21"])
    w = {
        "W11": W11r / std,
        "b11": _np32(inputs["b11"]) - (mean / std) * W11r.sum(0),
        "W21": W21r / std,
        "b21": _np32(inputs["b21"]) - (mean / std) * W21r.sum(0),
        "W12": _np32(inputs["W12"])[:, 0],
        "b12": float(np.asarray(inputs["b12"])[0]),
        "W22": _np32(inputs["W22"])[:, 0],
        "b22": float(np.asarray(inputs["b22"])[0]),
        "g11": _np32(inputs["g11"]), "be11": _np32(inputs["be11"]),
        "g21": _np32(inputs["g21"]), "be21": _np32(inputs["be21"]),
        "w30": float(np.asarray(inputs["W3"])[0, 0]),
        "w31": float(np.asarray(inputs["W3"])[1, 0]),
        "b3": float(np.asarray(inputs["b3"])[0]),
        "alpha": float(np.asarray(inputs["alpha"])[0]),
    }
    return w


def _tail_is_degenerate(w):
    """True iff v+1e-5 is provably inside (-10+m, -m) for all sigmoid outputs,
    which forces round(Tv/10) <= -1 -> T_idx clamps to 0 -> Ln == 64."""
    lo = w["b3"] + 1e-5 + min(w["w30"], 0.0) + min(w["w31"], 0.0)
    hi = w["b3"] + 1e-5 + max(w["w30"], 0.0) + max(w["w31"], 0.0)
    m = 1e-3
    return (lo > -10.0 + m) and (hi < -m) and w["alpha"] >= 0.0


def kernel(**inputs) -> np.ndarray:
    x = _np32(inputs["x"])
    assert x.shape == (B, T, N + 1)
    w = _fold_weights(inputs)
    ln_affine = not (np.all(w["g11"] == 1.0) and np.all(w["be11"] == 0.0)
                     and np.all(w["g21"] == 1.0) and np.all(w["be21"] == 0.0))
    general_tail = not _tail_is_degenerate(w)
    BPC = B // N_CORES
    built = _get_built(w, BPC, ln_affine, general_tail)

    # pad with ones-row at t=64 (feeds the matmul bias trick)
    xp = np.empty((B, T + 1, N + 1), np.float32)
    xp[:, :T, :] = x
    xp[:, T, :] = 1.0

    in_maps = [{"x": xp[c * BPC:(c + 1) * BPC]} for c in range(N_CORES)]
    res = run_bass_kernel_spmd(built.nc, in_maps, list(range(N_CORES)))
    out = np.concatenate([r["y"] for r in res.results], axis=0)
    return out.astype(np.float32)


if __name__ == "__main__":
    rng = np.random.default_rng(0)
    print("kernel module ok")


# revision 27
# speedup vs baseline: 5626.0627x; 5626.0627x over previous
"""Trainium2 Bass kernel for nn_Diffusion_Model (ragged_sequence).

Pure data-parallel: batch B=4096 sharded as 512 per NeuronCore across 8 cores.

Per-core design (token layout: partitions = node n, free = per-token values):
  - x is host-padded with a ones-row -> per-b lhsT [65,128] so one PE matmul
    computes both h = x@W11+b11 (cols 0..63) and the raw transpose
    xT = x.T (cols 64..127, via an identity block in the rhs).
  - LayerNorm stats via DVE bn_stats (even/odd chunk merge), relu via ACT
    with per-partition -mu bias, W12 dot via broadcast multiply + segmented
    reduce. rstd is factored out of the relu (g=1, beta=0 fast path; general
    gamma/beta handled with two extra passes).
  - branch 2 ("down" channel) collapses over n: one [65,512] staged matrix,
    ARCH-A style stats via ones-matmuls, broadcast back over partitions with
    a K=1 matmul.
  - Ragged geometric tail as a Horner scan along t (tensor_tensor_scan,
    a = q with per-segment reset columns), extraction of s[Ln-1] either as
    the last column (when the weights provably force Ln==64) or via
    tensor_mask_reduce per batch.
  - Final partition reduction with a ones-vector matmul.
"""
import sys
import numpy as np

sys.path.insert(0, "/opt/trn_rl_repo")

import concourse.bacc as bacc
import concourse.tile as tile
import concourse.mybir as mybir
from concourse.bass_utils import run_bass_kernel_spmd

dt = mybir.dt
Alu = mybir.AluOpType
Act = mybir.ActivationFunctionType
Ax = mybir.AxisListType

N_CORES = 8
B, T, N, H = 4096, 64, 128, 64
LN_EPS = 1e-5


def _np32(a):
    return np.ascontiguousarray(np.asarray(a, dtype=np.float32))


class _Built:
    pass


def _build(weights, BPC, ln_affine, general_tail, unroll=1):
    """Build the per-core Bass program. weights: dict of host-folded arrays."""
    NG = BPC // 8  # groups of 8 batches
    nc = bacc.Bacc("TRN2", target_bir_lowering=False, debug=False,
                   num_devices=N_CORES)

    x_in = nc.dram_tensor("x", [T + 1, BPC, N + 1], dt.float32,
                          kind="ExternalInput").ap()
    d_in = nc.dram_tensor("d", [T + 1, BPC], dt.float32,
                          kind="ExternalInput").ap()
    y_out = nc.dram_tensor("y", [BPC, 1], dt.float32,
                           kind="ExternalOutput").ap()

    # ---- inline constants ----
    W11, b11 = weights["W11"], weights["b11"]     # [64,64],[64]
    W21, b21 = weights["W21"], weights["b21"]
    W12, b12 = weights["W12"], float(weights["b12"])   # [64], scalar
    W22, b22 = weights["W22"], float(weights["b22"])
    g11, be11 = weights["g11"], weights["be11"]
    g21, be21 = weights["g21"], weights["be21"]
    w30, w31, b3 = (float(weights["w30"]), float(weights["w31"]),
                    float(weights["b3"]))
    alpha = float(weights["alpha"])

    # fold LN mean-centering into the weights (exact linear algebra):
    # mean_j of (x@W + b) = x@rowmean(W) + mean(b); subtracting it is the
    # same matmul with row-centered W and mean-centered b.
    W11c = (W11.astype(np.float64)
            - W11.astype(np.float64).mean(1, keepdims=True)).astype(np.float32)
    b11c = (b11.astype(np.float64) - b11.astype(np.float64).mean()).astype(np.float32)
    W21c = (W21.astype(np.float64)
            - W21.astype(np.float64).mean(1, keepdims=True)).astype(np.float32)
    b21c = (b21.astype(np.float64) - b21.astype(np.float64).mean()).astype(np.float32)
    RW = np.zeros((T + 1, 2 * H), np.float32)
    RW[:T, :H] = W11c
    RW[T, :H] = b11c
    RW[:T, H:] = np.eye(T, dtype=np.float32)
    RW2 = np.zeros((T + 1, H), np.float32)
    RW2[:T, :] = W21c
    RW2[T, :] = b21c

    w12bc = np.broadcast_to(W12[None, :], (128, H)).copy()
    g11bc = np.broadcast_to(g11[None, :], (128, H)).copy()
    be11bc = np.broadcast_to(be11[None, :], (128, H)).copy()
    t0m = np.ones((128, 512), np.float32)
    t0m[:, ::T] = 0.0
    ones_col = np.ones((128, 1), np.float32)
    ones64_col = np.ones((T, 1), np.float32)
    noneg64_row = np.full((1, H), -1.0 / H, np.float32)
    w22_col = W22.reshape(T, 1).astype(np.float32)

    c_RW = nc.inline_tensor(RW, "c_RW")
    c_RW2 = nc.inline_tensor(RW2, "c_RW2")
    c_w12bc = nc.inline_tensor(w12bc, "c_w12bc")
    c_t0m = nc.inline_tensor(t0m, "c_t0m")
    c_ones = nc.inline_tensor(ones_col, "c_ones")
    c_ones64 = nc.inline_tensor(ones64_col, "c_ones64")
    c_negmean = nc.inline_tensor(noneg64_row, "c_negmean")
    c_w22 = nc.inline_tensor(w22_col, "c_w22")
    if ln_affine:
        c_g11bc = nc.inline_tensor(g11bc, "c_g11bc")
        c_be11bc = nc.inline_tensor(be11bc, "c_be11bc")

    def _chunks(total, step=512):
        return [slice(i, min(i + step, total)) for i in range(0, total, step)]

    with tile.TileContext(nc) as tc:
        with tc.tile_pool(name="const", bufs=1) as cpool:
            RW_sb = cpool.tile([T + 1, 2 * H], dt.float32)
            nc.sync.dma_start(RW_sb[:], c_RW.ap())
            RW2_sb = cpool.tile([T + 1, H], dt.float32)
            nc.sync.dma_start(RW2_sb[:], c_RW2.ap())
            w12_sb = cpool.tile([128, H], dt.float32)
            nc.sync.dma_start(w12_sb[:], c_w12bc.ap())
            t0m_sb = cpool.tile([128, 512], dt.float32)
            nc.sync.dma_start(t0m_sb[:], c_t0m.ap())
            ones_sb = cpool.tile([128, 1], dt.float32)
            nc.sync.dma_start(ones_sb[:], c_ones.ap())
            ones64_sb = cpool.tile([T, 1], dt.float32)
            nc.sync.dma_start(ones64_sb[:], c_ones64.ap())
            negmean_sb = cpool.tile([1, H], dt.float32)
            nc.sync.dma_start(negmean_sb[:], c_negmean.ap())
            w22_sb = cpool.tile([T, 1], dt.float32)
            nc.sync.dma_start(w22_sb[:], c_w22.ap())
            if ln_affine:
                g11_sb = cpool.tile([128, H], dt.float32)
                nc.sync.dma_start(g11_sb[:], c_g11bc.ap())
                be11_sb = cpool.tile([128, H], dt.float32)
                nc.sync.dma_start(be11_sb[:], c_be11bc.ap())

            eps_sb = cpool.tile([128, 1], dt.float32)
            nc.vector.memset(eps_sb[:], LN_EPS)
            b22_sb = cpool.tile([128, 1], dt.float32)
            nc.vector.memset(b22_sb[:], b22)
            nb12_sb = cpool.tile([128, 1], dt.float32)
            nc.vector.memset(nb12_sb[:], -b12)
            nb22_sb = cpool.tile([128, 1], dt.float32)
            nc.vector.memset(nb22_sb[:], -b22)

            xdbc_sb = cpool.tile([128, BPC], dt.float32)   # xd broadcast
            acc_sb = cpool.tile([128, BPC], dt.float32)    # F * s* per token

            # ================= branch 2 (down channel), once =================
            with tc.tile_pool(name="b2", bufs=1) as b2, \
                 tc.tile_pool(name="b2ps", bufs=1, space="PSUM") as b2ps:
                onesrow = b2.tile([1, 128], dt.float32)
                nc.vector.memset(onesrow[:], 1.0)
                dstage = b2.tile([T + 1, BPC], dt.float32)
                nc.sync.dma_start(dstage[:], d_in[:])
                ps2c = b2ps.tile([H, BPC], dt.float32)
                for sl in _chunks(BPC):
                    nc.tensor.matmul(ps2c[:, sl], RW2_sb[:], dstage[:, sl],
                                     start=True, stop=True)
                sq2 = b2.tile([H, BPC], dt.float32)
                nc.scalar.square(sq2[:], ps2c[:])
                psS2 = b2ps.tile([1, BPC], dt.float32)
                for sl in _chunks(BPC):
                    nc.tensor.matmul(psS2[0:1, sl], ones64_sb[:], sq2[:, sl],
                                     start=True, stop=True)
                rl2 = b2.tile([H, BPC], dt.float32)
                if ln_affine:
                    # general gamma/beta for branch 2: nh*g+be then relu
                    lnv2r = b2.tile([1, BPC], dt.float32)
                    nc.scalar.activation(lnv2r[:], psS2[0:1, :], Act.Ln,
                                         scale=1.0 / H, bias=eps_sb[0:1, :])
                    rstd2r = b2.tile([1, BPC], dt.float32)
                    nc.scalar.activation(rstd2r[:], lnv2r[:], Act.Exp,
                                         scale=-0.5)
                    # nh = hc * rstd (bcast via K=1 matmul) ... then *g+be
                    psb = b2ps.tile([H, BPC], dt.float32)
                    for sl in _chunks(BPC):
                        nc.tensor.matmul(psb[:, sl], onesrow[:, 0:H],
                                         rstd2r[:, sl], start=True, stop=True)
                    rsb = b2.tile([H, BPC], dt.float32)
                    nc.vector.tensor_copy(rsb[:], psb[:])
                    nh2 = b2.tile([H, BPC], dt.float32)
                    nc.vector.tensor_tensor(nh2[:], ps2c[:], rsb[:], Alu.mult)
                    g2 = np.broadcast_to(g21[:, None], (H, 1)).copy()
                    be2 = np.broadcast_to(be21[:, None], (H, 1)).copy()
                    c_g2 = nc.inline_tensor(g2.astype(np.float32), "c_g2")
                    c_be2 = nc.inline_tensor(be2.astype(np.float32), "c_be2")
                    g2_sb = b2.tile([H, 1], dt.float32)
                    nc.sync.dma_start(g2_sb[:], c_g2.ap())
                    be2_sb = b2.tile([H, 1], dt.float32)
                    nc.sync.dma_start(be2_sb[:], c_be2.ap())
                    nc.vector.tensor_scalar(nh2[:], nh2[:], g2_sb[:],
                                            be2_sb[:], Alu.mult, Alu.add)
                    nc.scalar.activation(rl2[:], nh2[:], Act.Relu)
                else:
                    nc.scalar.activation(rl2[:], ps2c[:], Act.Relu)
                psD = b2ps.tile([1, BPC], dt.float32)
                for sl in _chunks(BPC):
                    nc.tensor.matmul(psD[0:1, sl], w22_sb[:], rl2[:, sl],
                                     start=True, stop=True)
                # xd = sigmoid(rstd2*dots2 + b22)  (fast path)
                #      sigmoid(dots2 + b22)        (affine path: rstd inside)
                xdrow = b2.tile([1, BPC], dt.float32)
                if ln_affine:
                    en2a = b2.tile([1, BPC], dt.float32)
                    nc.scalar.activation(en2a[:], psD[0:1, :], Act.Exp,
                                         scale=-1.0, bias=nb22_sb[0:1, :])
                    nc.vector.tensor_scalar(en2a[:], en2a[:], 1.0, None,
                                            Alu.add)
                    nc.vector.reciprocal(xdrow[:], en2a[:])
                else:
                    lnv2 = b2.tile([1, BPC], dt.float32)
                    nc.scalar.activation(lnv2[:], psS2[0:1, :], Act.Ln,
                                         scale=1.0 / H, bias=eps_sb[0:1, :])
                    rstd2 = b2.tile([1, BPC], dt.float32)
                    nc.scalar.activation(rstd2[:], lnv2[:], Act.Exp,
                                         scale=-0.5)
                    xin2 = b2.tile([1, BPC], dt.float32)
                    nc.vector.tensor_tensor(xin2[:], psD[0:1, :], rstd2[:],
                                            Alu.mult)
                    en2 = b2.tile([1, BPC], dt.float32)
                    nc.scalar.activation(en2[:], xin2[:], Act.Exp,
                                         scale=-1.0, bias=nb22_sb[0:1, :])
                    nc.vector.tensor_scalar(en2[:], en2[:], 1.0, None, Alu.add)
                    nc.vector.reciprocal(xdrow[:], en2[:])
                # broadcast xd over partitions: K=1 ones matmul
                psX = b2ps.tile([128, BPC], dt.float32)
                for sl in _chunks(BPC):
                    nc.tensor.matmul(psX[:, sl], onesrow[:], xdrow[:, sl],
                                     start=True, stop=True)
                nc.vector.tensor_copy(xdbc_sb[:], psX[:])

            # ================= main loop over groups of 8 b =================
            with tc.tile_pool(name="xt", bufs=4) as xpool, \
                 tc.tile_pool(name="psh", bufs=3, space="PSUM") as pshpool, \
                 tc.tile_pool(name="psx", bufs=3, space="PSUM") as psxpool, \
                 tc.tile_pool(name="big", bufs=4) as bigp, \
                 tc.tile_pool(name="sm", bufs=6) as smp:
                assert NG % 2 == 0
                for p_u in range(unroll * (NG // 2)):
                    p = p_u % (NG // 2)
                    # per-pair staging for 16-wide scalar chain
                    sqs = smp.tile([128, 16], dt.float32, tag="sqs")
                    dots = smp.tile([128, 16], dt.float32, tag="dots")
                    vx = smp.tile([128, 16], dt.float32, tag="vx")
                    nc.vector.tensor_scalar(vx[:],
                                            xdbc_sb[:, p * 16:(p + 1) * 16],
                                            w31, b3 + 1e-5, Alu.mult, Alu.add)
                    pss = []
                    xts = []
                    # -------- phase 1: per-group heavy ops --------
                    for k in range(2):
                        g = 2 * p + k
                        xt = xpool.tile([T + 1, 8 * (N + 1)], dt.float32)
                        nc.sync.dma_start(xt[:],
                                          x_in[:, g * 8:(g + 1) * 8, :])
                        ps_h = pshpool.tile([128, 512], dt.float32)
                        ps_x = psxpool.tile([128, 512], dt.float32)
                        for j in range(8):
                            lhs = xt[:, j * (N + 1): j * (N + 1) + N]
                            nc.tensor.matmul(ps_h[:, j * H:(j + 1) * H],
                                             lhs, RW_sb[:, 0:H], start=True,
                                             stop=True)
                            nc.tensor.matmul(ps_x[:, j * T:(j + 1) * T],
                                             lhs, RW_sb[:, H:2 * H],
                                             start=True, stop=True)
                        if k == 0:
                            xts.append(ps_x)
                        else:
                            xtr = bigp.tile([128, 512], dt.float32, tag="xtr")
                            nc.scalar.copy(xtr[:], ps_x[:])
                            xts.append(xtr)
                        sqv = bigp.tile([128, 512], dt.float32, tag="sqv")
                        nc.scalar.square(sqv[:], ps_h[:])
                        nc.vector.tensor_reduce(
                            sqs[:, k * 8:(k + 1) * 8],
                            sqv[:].rearrange("p (g t) -> p g t", g=8),
                            Ax.X, Alu.add)
                        if not ln_affine:
                            rl = bigp.tile([128, 512], dt.float32, tag="rl")
                            nc.scalar.activation(rl[:], ps_h[:], Act.Relu)
                            dotp = bigp.tile([128, 512], dt.float32,
                                             tag="dotp")
                            w12v = w12_sb[:].unsqueeze(1).broadcast_to(
                                [128, 8, H])
                            nc.gpsimd.tensor_tensor(
                                dotp[:].rearrange("p (g t) -> p g t", g=8),
                                rl[:].rearrange("p (g t) -> p g t", g=8),
                                w12v, Alu.mult)
                            nc.vector.tensor_reduce(
                                dots[:, k * 8:(k + 1) * 8],
                                dotp[:].rearrange("p (g t) -> p g t", g=8),
                                Ax.X, Alu.add)
                        pss.append(ps_h)

                    # -------- phase 2: rstd (and affine relu/dot) --------
                    lnv = smp.tile([128, 16], dt.float32, tag="lnv")
                    nc.scalar.activation(lnv[:], sqs[:], Act.Ln,
                                         scale=1.0 / H, bias=eps_sb[:])
                    rstd = smp.tile([128, 16], dt.float32, tag="rstd")
                    nc.scalar.activation(rstd[:], lnv[:], Act.Exp, scale=-0.5)
                    if ln_affine:
                        for k in range(2):
                            ps = pss[k]
                            nh = bigp.tile([128, 512], dt.float32, tag="nh")
                            nh3 = nh[:].rearrange("p (g t) -> p g t", g=8)
                            rst_b = rstd[:, k * 8:(k + 1) * 8].unsqueeze(
                                2).broadcast_to([128, 8, H])
                            nc.vector.tensor_tensor(
                                nh3,
                                ps[:].rearrange("p (g t) -> p g t", g=8),
                                rst_b, Alu.mult)
                            g_b = g11_sb[:].unsqueeze(1).broadcast_to(
                                [128, 8, H])
                            be_b = be11_sb[:].unsqueeze(1).broadcast_to(
                                [128, 8, H])
                            nc.vector.tensor_tensor(nh3, nh3, g_b, Alu.mult)
                            nc.vector.tensor_tensor(nh3, nh3, be_b, Alu.add)
                            rl = bigp.tile([128, 512], dt.float32, tag="rl")
                            nc.vector.tensor_scalar(rl[:], nh[:], 0.0, None,
                                                    Alu.max)
                            dotp = bigp.tile([128, 512], dt.float32,
                                             tag="dotp")
                            w12v = w12_sb[:].unsqueeze(1).broadcast_to(
                                [128, 8, H])
                            nc.gpsimd.tensor_tensor(
                                dotp[:].rearrange("p (g t) -> p g t", g=8),
                                rl[:].rearrange("p (g t) -> p g t", g=8),
                                w12v, Alu.mult)
                            nc.vector.tensor_reduce(
                                dots[:, k * 8:(k + 1) * 8],
                                dotp[:].rearrange("p (g t) -> p g t", g=8),
                                Ax.X, Alu.add)

                    # -------- phase 3: 16-wide scalar chain --------
                    xin = smp.tile([128, 16], dt.float32, tag="xin")
                    if ln_affine:
                        nc.vector.tensor_copy(xin[:], dots[:])
                    else:
                        nc.vector.tensor_tensor(xin[:], dots[:], rstd[:],
                                                Alu.mult)
                    exu = smp.tile([128, 16], dt.float32, tag="exu")
                    nc.scalar.activation(exu[:], xin[:], Act.Exp, scale=-1.0,
                                         bias=nb12_sb[:])
                    nc.vector.tensor_scalar(exu[:], exu[:], 1.0, None, Alu.add)
                    xu = smp.tile([128, 16], dt.float32, tag="xu")
                    nc.vector.reciprocal(xu[:], exu[:])
                    # v + 1e-5 = w30*xu + (w31*xd + b3 + 1e-5)
                    v1 = smp.tile([128, 16], dt.float32, tag="v1")
                    nc.vector.scalar_tensor_tensor(v1[:], xu[:], w30, vx[:],
                                                   Alu.mult, Alu.add)
                    rr = smp.tile([128, 16], dt.float32, tag="rr")
                    nc.vector.reciprocal(rr[:], v1[:])
                    fden = smp.tile([128, 16], dt.float32, tag="fden")
                    nc.vector.tensor_scalar(fden[:], rr[:], 50.0 * alpha, 1.0,
                                            Alu.mult, Alu.add)
                    F = smp.tile([128, 16], dt.float32, tag="F")
                    nc.vector.reciprocal(F[:], fden[:])
                    q = smp.tile([128, 16], dt.float32, tag="q")
                    nc.vector.tensor_scalar(q[:], F[:], -1.0, 1.0, Alu.mult,
                                            Alu.add)
                    if general_tail:
                        y5 = smp.tile([128, 16], dt.float32, tag="y5")
                        nc.vector.tensor_scalar(y5[:], rr[:], 5.0, 0.5,
                                                Alu.mult, Alu.add)
                        yi = smp.tile([128, 16], dt.int32, tag="yi")
                        nc.vector.tensor_copy(yi[:], y5[:])
                        yf = smp.tile([128, 16], dt.float32, tag="yf")
                        nc.vector.tensor_copy(yf[:], yi[:])
                        Tc = smp.tile([128, 16], dt.float32, tag="Tc")
                        nc.vector.tensor_scalar(Tc[:], yf[:], 0.0, 63.0,
                                                Alu.max, Alu.min)
                        mst = smp.tile([128, 16], dt.float32, tag="mst")
                        nc.vector.tensor_scalar(mst[:], Tc[:], -1.0, 63.0,
                                                Alu.mult, Alu.add)
                        men = smp.tile([128, 16], dt.float32, tag="men")
                        nc.vector.tensor_scalar(men[:], Tc[:], -1.0, 64.0,
                                                Alu.mult, Alu.add)

                    # -------- phase 4: per-group scan + accumulate --------
                    for k in range(2):
                        g = 2 * p + k
                        ksl = slice(k * 8, (k + 1) * 8)
                        a = bigp.tile([128, 512], dt.float32, tag="a")
                        qb = q[:, ksl].unsqueeze(2).broadcast_to([128, 8, T])
                        nc.gpsimd.affine_select(
                            a[:].rearrange("p (g t) -> p g t", g=8), qb,
                            pattern=[[0, 8], [1, T]], compare_op=Alu.is_gt,
                            fill=0.0, base=0, channel_multiplier=0)
                        s = bigp.tile([128, 512], dt.float32, tag="s")
                        nc.vector.tensor_tensor_scan(
                            s[:], a[:], xts[k][:], 0.0, Alu.mult, Alu.add)
                        accs = acc_sb[:, g * 8:(g + 1) * 8]
                        if general_tail:
                            sstar = smp.tile([128, 8], dt.float32, tag="sstar")
                            junk = bigp.tile([128, 64], dt.float32, tag="junk")
                            for j in range(8):
                                nc.vector.tensor_mask_reduce(
                                    junk[:], s[:, j * T:(j + 1) * T],
                                    mst[:, k * 8 + j:k * 8 + j + 1],
                                    men[:, k * 8 + j:k * 8 + j + 1], 1.0,
                                    -3.0e38, Alu.max,
                                    accum_out=sstar[:, j:j + 1])
                            nc.vector.tensor_tensor(accs, sstar[:], F[:, ksl],
                                                    Alu.mult)
                        else:
                            slast = s[:].rearrange(
                                "p (g t) -> p g t", g=8)[:, :, T - 1]
                            nc.vector.tensor_tensor(accs, slast, F[:, ksl],
                                                    Alu.mult)

                # ---- final: pred[b] = sum over partitions of acc ----
                with tc.tile_pool(name="fin", bufs=1) as fin, \
                     tc.tile_pool(name="finps", bufs=1, space="PSUM") as fps:
                    po = fps.tile([1, BPC], dt.float32)
                    for sl in _chunks(BPC):
                        nc.tensor.matmul(po[0:1, sl], ones_sb[:],
                                         acc_sb[:, sl], start=True, stop=True)
                    pred = fin.tile([1, BPC], dt.float32)
                    nc.vector.tensor_copy(pred[:], po[0:1, :])
                    nc.sync.dma_start(
                        y_out.rearrange("b one -> one b"), pred[:])

    # Force all activations onto the one table set that contains every
    # function we use (Relu/Square/Ln/Exp/Copy/Identity), so the compiled
    # stream has a single ACT table load instead of per-group thrash.
    # The pass picks the first listed set containing each function; ids must
    # stay aligned with act_info.json order, so empty out the other sets.
    import types
    from concourse.hw_specs import get_activation_tables
    import concourse._compat as _cc
    orig_tables = list(get_activation_tables(nc.m.arch).items())
    patched_tables = [
        (name, s if name == "natural_log_exp_and_others" else set())
        for name, s in orig_tables
    ]
    import bass_rust as _bass_rust_mod

    def _patched_act_loads(self):
        has_activation = any(
            type(i).__name__ == "InstActivation"
            for b in self.main_func.blocks
            for i in b.instructions
        )
        if not has_activation:
            return
        _bass_rust_mod.insert_act_table_loads(self, patched_tables)

    nc.insert_act_table_loads = types.MethodType(_patched_act_loads, nc)

    nc.compile()
    built = _Built()
    built.nc = nc
    built.BPC = BPC
    return built


_CACHE = {}


def _get_built(weights, BPC, ln_affine, general_tail):
    key = (BPC, ln_affine, general_tail,
           hash(tuple(np.asarray(v).tobytes() for v in (
               weights["W11"].ravel()[:4], weights["b11"].ravel()[:4]))))
    # cache on full weight bytes to be safe
    full_key = (BPC, ln_affine, general_tail,
                b"".join(_np32(weights[k]).tobytes() for k in sorted(weights)))
    if full_key not in _CACHE:
        _CACHE[full_key] = _build(weights, BPC, ln_affine, general_tail)
    return _CACHE[full_key]


def _fold_weights(inputs):
    mean = float(np.asarray(inputs["x_mean"]))
    std = float(np.asarray(inputs["x_std"]))
    W11r = _np32(inputs["W11"])
    W21r = _np32(inputs["W21"])
    w = {
        "W11": W11r / std,
        "b11": _np32(inputs["b11"]) - (mean / std) * W11r.sum(0),
        "W21": W21r / std,
        "b21": _np32(inputs["b21"]) - (mean / std) * W21r.sum(0),
        "W12": _np32(inputs["W12"])[:, 0],
        "b12": float(np.asarray(inputs["b12"])[0]),
        "W22": _np32(inputs["W22"])[:, 0],
        "b22": float(np.asarray(inputs["b22"])[0]),
        "g11": _np32(inputs["g11"]), "be11": _np32(inputs["be11"]),
        "g21": _np32(inputs["g21"]), "be21": _np32(inputs["be21"]),
        "w30": float(np.asarray(inputs["W3"])[0, 0]),
        "w31": float(np.asarray(inputs["W3"])[1, 0]),
        "b3": float(np.asarray(inputs["b3"])[0]),
        "alpha": float(np.asarray(inputs["alpha"])[0]),
    }
    return w


def _tail_is_degenerate(w):
    """True iff v+1e-5 is provably inside (-10+m, -m) for all sigmoid outputs,
    which forces round(Tv/10) <= -1 -> T_idx clamps to 0 -> Ln == 64."""
    lo = w["b3"] + 1e-5 + min(w["w30"], 0.0) + min(w["w31"], 0.0)
    hi = w["b3"] + 1e-5 + max(w["w30"], 0.0) + max(w["w31"], 0.0)
    m = 1e-3
    return (lo > -10.0 + m) and (hi < -m) and w["alpha"] >= 0.0


def kernel(**inputs) -> np.ndarray:
    x = _np32(inputs["x"])
    assert x.shape == (B, T, N + 1)
    w = _fold_weights(inputs)
    ln_affine = not (np.all(w["g11"] == 1.0) and np.all(w["be11"] == 0.0)
                     and np.all(w["g21"] == 1.0) and np.all(w["be21"] == 0.0))
    general_tail = not _tail_is_degenerate(w)
    BPC = B // N_CORES
    built = _get_built(w, BPC, ln_affine, general_tail)

    # per-core t-major layout with a ones-row at t=64 (matmul bias trick)
    in_maps = []
    for c in range(N_CORES):
        xs = x[c * BPC:(c + 1) * BPC]          # [BPC, T, N+1]
        xp = np.empty((T + 1, BPC, N + 1), np.float32)
        xp[:T] = xs.transpose(1, 0, 2)
        xp[T] = 1.0
        in_maps.append({"x": xp, "d": np.ascontiguousarray(xp[:, :, N])})
    res = run_bass_kernel_spmd(built.nc, in_maps, list(range(N_CORES)))
    out = np.concatenate([r["y"] for r in res.results], axis=0)
    return out.astype(np.float32)


if __name__ == "__main__":
    rng = np.random.default_rng(0)
    print("kernel module ok")


# revision 28
# speedup vs baseline: 8567.5299x; 1.5228x over previous
"""Trainium2 Bass kernel for nn_Diffusion_Model (ragged_sequence).

Pure data-parallel: batch B=4096 sharded as 512 per NeuronCore across 8 cores.

Per-core design (token layout: partitions = node n, free = per-token values):
  - x is host-padded with a ones-row -> per-b lhsT [65,128] so one PE matmul
    computes both h = x@W11+b11 (cols 0..63) and the raw transpose
    xT = x.T (cols 64..127, via an identity block in the rhs).
  - LayerNorm stats via DVE bn_stats (even/odd chunk merge), relu via ACT
    with per-partition -mu bias, W12 dot via broadcast multiply + segmented
    reduce. rstd is factored out of the relu (g=1, beta=0 fast path; general
    gamma/beta handled with two extra passes).
  - branch 2 ("down" channel) collapses over n: one [65,512] staged matrix,
    ARCH-A style stats via ones-matmuls, broadcast back over partitions with
    a K=1 matmul.
  - Ragged geometric tail as a Horner scan along t (tensor_tensor_scan,
    a = q with per-segment reset columns), extraction of s[Ln-1] either as
    the last column (when the weights provably force Ln==64) or via
    tensor_mask_reduce per batch.
  - Final partition reduction with a ones-vector matmul.
"""
import sys
import numpy as np

sys.path.insert(0, "/opt/trn_rl_repo")

import concourse.bacc as bacc
import concourse.tile as tile
import concourse.mybir as mybir
from concourse.bass_utils import run_bass_kernel_spmd

dt = mybir.dt
Alu = mybir.AluOpType
Act = mybir.ActivationFunctionType
Ax = mybir.AxisListType

N_CORES = 8
B, T, N, H = 4096, 64, 128, 64
LN_EPS = 1e-5


def _np32(a):
    return np.ascontiguousarray(np.asarray(a, dtype=np.float32))


class _Built:
    pass


def _build(weights, BPC, ln_affine, general_tail, unroll=1):
    """Build the per-core Bass program. weights: dict of host-folded arrays."""
    NG = BPC // 8  # groups of 8 batches
    nc = bacc.Bacc("TRN2", target_bir_lowering=False, debug=False,
                   num_devices=N_CORES)

    x_in = nc.dram_tensor("x", [T + 1, BPC, N + 1], dt.float32,
                          kind="ExternalInput").ap()
    d_in = nc.dram_tensor("d", [T + 1, BPC], dt.float32,
                          kind="ExternalInput").ap()
    y_out = nc.dram_tensor("y", [BPC, 1], dt.float32,
                           kind="ExternalOutput").ap()

    # ---- inline constants ----
    W11, b11 = weights["W11"], weights["b11"]     # [64,64],[64]
    W21, b21 = weights["W21"], weights["b21"]
    W12, b12 = weights["W12"], float(weights["b12"])   # [64], scalar
    W22, b22 = weights["W22"], float(weights["b22"])
    g11, be11 = weights["g11"], weights["be11"]
    g21, be21 = weights["g21"], weights["be21"]
    w30, w31, b3 = (float(weights["w30"]), float(weights["w31"]),
                    float(weights["b3"]))
    alpha = float(weights["alpha"])

    # fold LN mean-centering into the weights (exact linear algebra):
    # mean_j of (x@W + b) = x@rowmean(W) + mean(b); subtracting it is the
    # same matmul with row-centered W and mean-centered b.
    W11c = (W11.astype(np.float64)
            - W11.astype(np.float64).mean(1, keepdims=True)).astype(np.float32)
    b11c = (b11.astype(np.float64) - b11.astype(np.float64).mean()).astype(np.float32)
    W21c = (W21.astype(np.float64)
            - W21.astype(np.float64).mean(1, keepdims=True)).astype(np.float32)
    b21c = (b21.astype(np.float64) - b21.astype(np.float64).mean()).astype(np.float32)
    RW = np.zeros((T + 1, 2 * H), np.float32)
    RW[:T, :H] = W11c
    RW[T, :H] = b11c
    RW[:T, H:] = np.eye(T, dtype=np.float32)
    RW2 = np.zeros((T + 1, H), np.float32)
    RW2[:T, :] = W21c
    RW2[T, :] = b21c

    w12bc = np.broadcast_to(W12[None, :], (128, H)).copy()
    g11bc = np.broadcast_to(g11[None, :], (128, H)).copy()
    be11bc = np.broadcast_to(be11[None, :], (128, H)).copy()
    t0m = np.ones((128, 512), np.float32)
    t0m[:, ::T] = 0.0
    ones_col = np.ones((128, 1), np.float32)
    ones64_col = np.ones((T, 1), np.float32)
    noneg64_row = np.full((1, H), -1.0 / H, np.float32)
    w22_col = W22.reshape(T, 1).astype(np.float32)

    c_RW = nc.inline_tensor(RW, "c_RW")
    c_RW2 = nc.inline_tensor(RW2, "c_RW2")
    c_w12bc = nc.inline_tensor(w12bc, "c_w12bc")
    c_t0m = nc.inline_tensor(t0m, "c_t0m")
    c_ones = nc.inline_tensor(ones_col, "c_ones")
    c_ones64 = nc.inline_tensor(ones64_col, "c_ones64")
    c_negmean = nc.inline_tensor(noneg64_row, "c_negmean")
    c_w22 = nc.inline_tensor(w22_col, "c_w22")
    if ln_affine:
        c_g11bc = nc.inline_tensor(g11bc, "c_g11bc")
        c_be11bc = nc.inline_tensor(be11bc, "c_be11bc")

    def _chunks(total, step=512):
        return [slice(i, min(i + step, total)) for i in range(0, total, step)]

    with tile.TileContext(nc) as tc:
        with tc.tile_pool(name="const", bufs=1) as cpool:
            RW_sb = cpool.tile([T + 1, 2 * H], dt.float32)
            nc.sync.dma_start(RW_sb[:], c_RW.ap())
            RW2_sb = cpool.tile([T + 1, H], dt.float32)
            nc.sync.dma_start(RW2_sb[:], c_RW2.ap())
            w12_sb = cpool.tile([128, H], dt.float32)
            nc.sync.dma_start(w12_sb[:], c_w12bc.ap())
            t0m_sb = cpool.tile([128, 512], dt.float32)
            nc.sync.dma_start(t0m_sb[:], c_t0m.ap())
            ones_sb = cpool.tile([128, 1], dt.float32)
            nc.sync.dma_start(ones_sb[:], c_ones.ap())
            ones64_sb = cpool.tile([T, 1], dt.float32)
            nc.sync.dma_start(ones64_sb[:], c_ones64.ap())
            negmean_sb = cpool.tile([1, H], dt.float32)
            nc.sync.dma_start(negmean_sb[:], c_negmean.ap())
            w22_sb = cpool.tile([T, 1], dt.float32)
            nc.sync.dma_start(w22_sb[:], c_w22.ap())
            if ln_affine:
                g11_sb = cpool.tile([128, H], dt.float32)
                nc.sync.dma_start(g11_sb[:], c_g11bc.ap())
                be11_sb = cpool.tile([128, H], dt.float32)
                nc.sync.dma_start(be11_sb[:], c_be11bc.ap())

            eps_sb = cpool.tile([128, 1], dt.float32)
            nc.vector.memset(eps_sb[:], LN_EPS)
            b22_sb = cpool.tile([128, 1], dt.float32)
            nc.vector.memset(b22_sb[:], b22)
            nb12_sb = cpool.tile([128, 1], dt.float32)
            nc.vector.memset(nb12_sb[:], -b12)
            nb22_sb = cpool.tile([128, 1], dt.float32)
            nc.vector.memset(nb22_sb[:], -b22)

            xdbc_sb = cpool.tile([128, BPC], dt.float32)   # xd broadcast
            acc_sb = cpool.tile([128, BPC], dt.float32)    # F * s* per token

            # ================= branch 2 (down channel), once =================
            with tc.tile_pool(name="b2", bufs=1) as b2, \
                 tc.tile_pool(name="b2ps", bufs=1, space="PSUM") as b2ps:
                onesrow = b2.tile([1, 128], dt.float32)
                nc.vector.memset(onesrow[:], 1.0)
                dstage = b2.tile([T + 1, BPC], dt.float32)
                nc.sync.dma_start(dstage[:], d_in[:])
                ps2c = b2ps.tile([H, BPC], dt.float32)
                for sl in _chunks(BPC):
                    nc.tensor.matmul(ps2c[:, sl], RW2_sb[:], dstage[:, sl],
                                     start=True, stop=True)
                sq2 = b2.tile([H, BPC], dt.float32)
                nc.scalar.square(sq2[:], ps2c[:])
                psS2 = b2ps.tile([1, BPC], dt.float32)
                for sl in _chunks(BPC):
                    nc.tensor.matmul(psS2[0:1, sl], ones64_sb[:], sq2[:, sl],
                                     start=True, stop=True)
                rl2 = b2.tile([H, BPC], dt.float32)
                if ln_affine:
                    # general gamma/beta for branch 2: nh*g+be then relu
                    lnv2r = b2.tile([1, BPC], dt.float32)
                    nc.scalar.activation(lnv2r[:], psS2[0:1, :], Act.Ln,
                                         scale=1.0 / H, bias=eps_sb[0:1, :])
                    rstd2r = b2.tile([1, BPC], dt.float32)
                    nc.scalar.activation(rstd2r[:], lnv2r[:], Act.Exp,
                                         scale=-0.5)
                    # nh = hc * rstd (bcast via K=1 matmul) ... then *g+be
                    psb = b2ps.tile([H, BPC], dt.float32)
                    for sl in _chunks(BPC):
                        nc.tensor.matmul(psb[:, sl], onesrow[:, 0:H],
                                         rstd2r[:, sl], start=True, stop=True)
                    rsb = b2.tile([H, BPC], dt.float32)
                    nc.vector.tensor_copy(rsb[:], psb[:])
                    nh2 = b2.tile([H, BPC], dt.float32)
                    nc.vector.tensor_tensor(nh2[:], ps2c[:], rsb[:], Alu.mult)
                    g2 = np.broadcast_to(g21[:, None], (H, 1)).copy()
                    be2 = np.broadcast_to(be21[:, None], (H, 1)).copy()
                    c_g2 = nc.inline_tensor(g2.astype(np.float32), "c_g2")
                    c_be2 = nc.inline_tensor(be2.astype(np.float32), "c_be2")
                    g2_sb = b2.tile([H, 1], dt.float32)
                    nc.sync.dma_start(g2_sb[:], c_g2.ap())
                    be2_sb = b2.tile([H, 1], dt.float32)
                    nc.sync.dma_start(be2_sb[:], c_be2.ap())
                    nc.vector.tensor_scalar(nh2[:], nh2[:], g2_sb[:],
                                            be2_sb[:], Alu.mult, Alu.add)
                    nc.scalar.activation(rl2[:], nh2[:], Act.Relu)
                else:
                    nc.scalar.activation(rl2[:], ps2c[:], Act.Relu)
                psD = b2ps.tile([1, BPC], dt.float32)
                for sl in _chunks(BPC):
                    nc.tensor.matmul(psD[0:1, sl], w22_sb[:], rl2[:, sl],
                                     start=True, stop=True)
                # xd = sigmoid(rstd2*dots2 + b22)  (fast path)
                #      sigmoid(dots2 + b22)        (affine path: rstd inside)
                xdrow = b2.tile([1, BPC], dt.float32)
                if ln_affine:
                    en2a = b2.tile([1, BPC], dt.float32)
                    nc.scalar.activation(en2a[:], psD[0:1, :], Act.Exp,
                                         scale=-1.0, bias=nb22_sb[0:1, :])
                    nc.vector.tensor_scalar(en2a[:], en2a[:], 1.0, None,
                                            Alu.add)
                    nc.vector.reciprocal(xdrow[:], en2a[:])
                else:
                    lnv2 = b2.tile([1, BPC], dt.float32)
                    nc.scalar.activation(lnv2[:], psS2[0:1, :], Act.Ln,
                                         scale=1.0 / H, bias=eps_sb[0:1, :])
                    rstd2 = b2.tile([1, BPC], dt.float32)
                    nc.scalar.activation(rstd2[:], lnv2[:], Act.Exp,
                                         scale=-0.5)
                    xin2 = b2.tile([1, BPC], dt.float32)
                    nc.vector.tensor_tensor(xin2[:], psD[0:1, :], rstd2[:],
                                            Alu.mult)
                    en2 = b2.tile([1, BPC], dt.float32)
                    nc.scalar.activation(en2[:], xin2[:], Act.Exp,
                                         scale=-1.0, bias=nb22_sb[0:1, :])
                    nc.vector.tensor_scalar(en2[:], en2[:], 1.0, None, Alu.add)
                    nc.vector.reciprocal(xdrow[:], en2[:])
                # broadcast xd over partitions: K=1 ones matmul
                psX = b2ps.tile([128, BPC], dt.float32)
                for sl in _chunks(BPC):
                    nc.tensor.matmul(psX[:, sl], onesrow[:], xdrow[:, sl],
                                     start=True, stop=True)
                nc.vector.tensor_copy(xdbc_sb[:], psX[:])

            # ================= main loop over groups of 8 b =================
            with tc.tile_pool(name="xt", bufs=4) as xpool, \
                 tc.tile_pool(name="psh", bufs=3, space="PSUM") as pshpool, \
                 tc.tile_pool(name="psx", bufs=3, space="PSUM") as psxpool, \
                 tc.tile_pool(name="big", bufs=4) as bigp, \
                 tc.tile_pool(name="sm", bufs=6) as smp:
                assert NG % 2 == 0
                for p_u in range(unroll * (NG // 2)):
                    p = p_u % (NG // 2)
                    # per-pair staging for 16-wide scalar chain
                    sqs = smp.tile([128, 16], dt.float32, tag="sqs")
                    dots = smp.tile([128, 16], dt.float32, tag="dots")
                    vx = smp.tile([128, 16], dt.float32, tag="vx")
                    nc.vector.tensor_scalar(vx[:],
                                            xdbc_sb[:, p * 16:(p + 1) * 16],
                                            w31, b3 + 1e-5, Alu.mult, Alu.add)
                    pss = []
                    xts = []
                    # -------- phase 1: per-group heavy ops --------
                    for k in range(2):
                        g = 2 * p + k
                        xt = xpool.tile([T + 1, 8 * (N + 1)], dt.float32)
                        nc.sync.dma_start(xt[:],
                                          x_in[:, g * 8:(g + 1) * 8, :])
                        ps_h = pshpool.tile([128, 512], dt.float32)
                        ps_x = psxpool.tile([128, 512], dt.float32)
                        for j in range(8):
                            lhs = xt[:, j * (N + 1): j * (N + 1) + N]
                            nc.tensor.matmul(ps_h[:, j * H:(j + 1) * H],
                                             lhs, RW_sb[:, 0:H], start=True,
                                             stop=True)
                            nc.tensor.matmul(ps_x[:, j * T:(j + 1) * T],
                                             lhs, RW_sb[:, H:2 * H],
                                             start=True, stop=True)
                        if k == 0:
                            xts.append(ps_x)
                        else:
                            xtr = bigp.tile([128, 512], dt.float32, tag="xtr")
                            nc.scalar.copy(xtr[:], ps_x[:])
                            xts.append(xtr)
                        sqv = bigp.tile([128, 512], dt.float32, tag="sqv")
                        nc.scalar.square(sqv[:], ps_h[:])
                        nc.vector.tensor_reduce(
                            sqs[:, k * 8:(k + 1) * 8],
                            sqv[:].rearrange("p (g t) -> p g t", g=8),
                            Ax.X, Alu.add)
                        if not ln_affine:
                            rl = bigp.tile([128, 512], dt.float32, tag="rl")
                            nc.scalar.activation(rl[:], ps_h[:], Act.Relu)
                            dotp = bigp.tile([128, 512], dt.float32,
                                             tag="dotp")
                            w12v = w12_sb[:].unsqueeze(1).broadcast_to(
                                [128, 8, H])
                            nc.vector.tensor_tensor(
                                dotp[:].rearrange("p (g t) -> p g t", g=8),
                                rl[:].rearrange("p (g t) -> p g t", g=8),
                                w12v, Alu.mult)
                            nc.vector.tensor_reduce(
                                dots[:, k * 8:(k + 1) * 8],
                                dotp[:].rearrange("p (g t) -> p g t", g=8),
                                Ax.X, Alu.add)
                        pss.append(ps_h)

                    # -------- phase 2: rstd (and affine relu/dot) --------
                    lnv = smp.tile([128, 16], dt.float32, tag="lnv")
                    nc.scalar.activation(lnv[:], sqs[:], Act.Ln,
                                         scale=1.0 / H, bias=eps_sb[:])
                    rstd = smp.tile([128, 16], dt.float32, tag="rstd")
                    nc.scalar.activation(rstd[:], lnv[:], Act.Exp, scale=-0.5)
                    if ln_affine:
                        for k in range(2):
                            ps = pss[k]
                            nh = bigp.tile([128, 512], dt.float32, tag="nh")
                            nh3 = nh[:].rearrange("p (g t) -> p g t", g=8)
                            rst_b = rstd[:, k * 8:(k + 1) * 8].unsqueeze(
                                2).broadcast_to([128, 8, H])
                            nc.vector.tensor_tensor(
                                nh3,
                                ps[:].rearrange("p (g t) -> p g t", g=8),
                                rst_b, Alu.mult)
                            g_b = g11_sb[:].unsqueeze(1).broadcast_to(
                                [128, 8, H])
                            be_b = be11_sb[:].unsqueeze(1).broadcast_to(
                                [128, 8, H])
                            nc.vector.tensor_tensor(nh3, nh3, g_b, Alu.mult)
                            nc.vector.tensor_tensor(nh3, nh3, be_b, Alu.add)
                            rl = bigp.tile([128, 512], dt.float32, tag="rl")
                            nc.vector.tensor_scalar(rl[:], nh[:], 0.0, None,
                                                    Alu.max)
                            dotp = bigp.tile([128, 512], dt.float32,
                                             tag="dotp")
                            w12v = w12_sb[:].unsqueeze(1).broadcast_to(
                                [128, 8, H])
                            nc.gpsimd.tensor_tensor(
                                dotp[:].rearrange("p (g t) -> p g t", g=8),
                                rl[:].rearrange("p (g t) -> p g t", g=8),
                                w12v, Alu.mult)
                            nc.vector.tensor_reduce(
                                dots[:, k * 8:(k + 1) * 8],
                                dotp[:].rearrange("p (g t) -> p g t", g=8),
                                Ax.X, Alu.add)

                    # -------- phase 3: 16-wide scalar chain --------
                    xin = smp.tile([128, 16], dt.float32, tag="xin")
                    if ln_affine:
                        nc.vector.tensor_copy(xin[:], dots[:])
                    else:
                        nc.vector.tensor_tensor(xin[:], dots[:], rstd[:],
                                                Alu.mult)
                    exu = smp.tile([128, 16], dt.float32, tag="exu")
                    nc.scalar.activation(exu[:], xin[:], Act.Exp, scale=-1.0,
                                         bias=nb12_sb[:])
                    nc.vector.tensor_scalar(exu[:], exu[:], 1.0, None, Alu.add)
                    xu = smp.tile([128, 16], dt.float32, tag="xu")
                    nc.vector.reciprocal(xu[:], exu[:])
                    # v + 1e-5 = w30*xu + (w31*xd + b3 + 1e-5)
                    v1 = smp.tile([128, 16], dt.float32, tag="v1")
                    nc.vector.scalar_tensor_tensor(v1[:], xu[:], w30, vx[:],
                                                   Alu.mult, Alu.add)
                    rr = smp.tile([128, 16], dt.float32, tag="rr")
                    nc.vector.reciprocal(rr[:], v1[:])
                    fden = smp.tile([128, 16], dt.float32, tag="fden")
                    nc.vector.tensor_scalar(fden[:], rr[:], 50.0 * alpha, 1.0,
                                            Alu.mult, Alu.add)
                    F = smp.tile([128, 16], dt.float32, tag="F")
                    nc.vector.reciprocal(F[:], fden[:])
                    q = smp.tile([128, 16], dt.float32, tag="q")
                    nc.vector.tensor_scalar(q[:], F[:], -1.0, 1.0, Alu.mult,
                                            Alu.add)
                    if general_tail:
                        y5 = smp.tile([128, 16], dt.float32, tag="y5")
                        nc.vector.tensor_scalar(y5[:], rr[:], 5.0, 0.5,
                                                Alu.mult, Alu.add)
                        yi = smp.tile([128, 16], dt.int32, tag="yi")
                        nc.vector.tensor_copy(yi[:], y5[:])
                        yf = smp.tile([128, 16], dt.float32, tag="yf")
                        nc.vector.tensor_copy(yf[:], yi[:])
                        Tc = smp.tile([128, 16], dt.float32, tag="Tc")
                        nc.vector.tensor_scalar(Tc[:], yf[:], 0.0, 63.0,
                                                Alu.max, Alu.min)
                        mst = smp.tile([128, 16], dt.float32, tag="mst")
                        nc.vector.tensor_scalar(mst[:], Tc[:], -1.0, 63.0,
                                                Alu.mult, Alu.add)
                        men = smp.tile([128, 16], dt.float32, tag="men")
                        nc.vector.tensor_scalar(men[:], Tc[:], -1.0, 64.0,
                                                Alu.mult, Alu.add)

                    # -------- phase 4: per-group scan + accumulate --------
                    for k in range(2):
                        g = 2 * p + k
                        ksl = slice(k * 8, (k + 1) * 8)
                        a = bigp.tile([128, 512], dt.float32, tag="a")
                        qb = q[:, ksl].unsqueeze(2).broadcast_to([128, 8, T])
                        nc.vector.tensor_tensor(
                            a[:].rearrange("p (g t) -> p g t", g=8), qb,
                            t0m_sb[:].rearrange("p (g t) -> p g t", g=8),
                            Alu.mult)
                        s = bigp.tile([128, 512], dt.float32, tag="s")
                        nc.vector.tensor_tensor_scan(
                            s[:], a[:], xts[k][:], 0.0, Alu.mult, Alu.add)
                        accs = acc_sb[:, g * 8:(g + 1) * 8]
                        if general_tail:
                            sstar = smp.tile([128, 8], dt.float32, tag="sstar")
                            junk = bigp.tile([128, 64], dt.float32, tag="junk")
                            for j in range(8):
                                nc.vector.tensor_mask_reduce(
                                    junk[:], s[:, j * T:(j + 1) * T],
                                    mst[:, k * 8 + j:k * 8 + j + 1],
                                    men[:, k * 8 + j:k * 8 + j + 1], 1.0,
                                    -3.0e38, Alu.max,
                                    accum_out=sstar[:, j:j + 1])
                            nc.vector.tensor_tensor(accs, sstar[:], F[:, ksl],
                                                    Alu.mult)
                        else:
                            slast = s[:].rearrange(
                                "p (g t) -> p g t", g=8)[:, :, T - 1]
                            nc.vector.tensor_tensor(accs, slast, F[:, ksl],
                                                    Alu.mult)

                # ---- final: pred[b] = sum over partitions of acc ----
                with tc.tile_pool(name="fin", bufs=1) as fin, \
                     tc.tile_pool(name="finps", bufs=1, space="PSUM") as fps:
                    po = fps.tile([1, BPC], dt.float32)
                    for sl in _chunks(BPC):
                        nc.tensor.matmul(po[0:1, sl], ones_sb[:],
                                         acc_sb[:, sl], start=True, stop=True)
                    pred = fin.tile([1, BPC], dt.float32)
                    nc.vector.tensor_copy(pred[:], po[0:1, :])
                    nc.sync.dma_start(
                        y_out.rearrange("b one -> one b"), pred[:])

    # Force all activations onto the one table set that contains every
    # function we use (Relu/Square/Ln/Exp/Copy/Identity), so the compiled
    # stream has a single ACT table load instead of per-group thrash.
    # The pass picks the first listed set containing each function; ids must
    # stay aligned with act_info.json order, so empty out the other sets.
    import types
    from concourse.hw_specs import get_activation_tables
    import concourse._compat as _cc
    orig_tables = list(get_activation_tables(nc.m.arch).items())
    patched_tables = [
        (name, s if name == "natural_log_exp_and_others" else set())
        for name, s in orig_tables
    ]
    import bass_rust as _bass_rust_mod

    def _patched_act_loads(self):
        has_activation = any(
            type(i).__name__ == "InstActivation"
            for b in self.main_func.blocks
            for i in b.instructions
        )
        if not has_activation:
            return
        _bass_rust_mod.insert_act_table_loads(self, patched_tables)

    nc.insert_act_table_loads = types.MethodType(_patched_act_loads, nc)

    nc.compile()
    built = _Built()
    built.nc = nc
    built.BPC = BPC
    return built


_CACHE = {}


def _get_built(weights, BPC, ln_affine, general_tail):
    key = (BPC, ln_affine, general_tail,
           hash(tuple(np.asarray(v).tobytes() for v in (
               weights["W11"].ravel()[:4], weights["b11"].ravel()[:4]))))
    # cache on full weight bytes to be safe
    full_key = (BPC, ln_affine, general_tail,
                b"".join(_np32(weights[k]).tobytes() for k in sorted(weights)))
    if full_key not in _CACHE:
        _CACHE[full_key] = _build(weights, BPC, ln_affine, general_tail)
    return _CACHE[full_key]


def _fold_weights(inputs):
    mean = float(np.asarray(inputs["x_mean"]))
    std = float(np.asarray(inputs["x_std"]))
    W11r = _np32(inputs["W11"])
    W21r = _np32(inputs["W21"])
    w = {
        "W11": W11r / std,
        "b11": _np32(inputs["b11"]) - (mean / std) * W11r.sum(0),
        "W21": W21r / std,
        "b21": _np32(inputs["b21"]) - (mean / std) * W21r.sum(0),
        "W12": _np32(inputs["W12"])[:, 0],
        "b12": float(np.asarray(inputs["b12"])[0]),
        "W22": _np32(inputs["W22"])[:, 0],
        "b22": float(np.asarray(inputs["b22"])[0]),
        "g11": _np32(inputs["g11"]), "be11": _np32(inputs["be11"]),
        "g21": _np32(inputs["g21"]), "be21": _np32(inputs["be21"]),
        "w30": float(np.asarray(inputs["W3"])[0, 0]),
        "w31": float(np.asarray(inputs["W3"])[1, 0]),
        "b3": float(np.asarray(inputs["b3"])[0]),
        "alpha": float(np.asarray(inputs["alpha"])[0]),
    }
    return w


def _tail_is_degenerate(w):
    """True iff v+1e-5 is provably inside (-10+m, -m) for all sigmoid outputs,
    which forces round(Tv/10) <= -1 -> T_idx clamps to 0 -> Ln == 64."""
    lo = w["b3"] + 1e-5 + min(w["w30"], 0.0) + min(w["w31"], 0.0)
    hi = w["b3"] + 1e-5 + max(w["w30"], 0.0) + max(w["w31"], 0.0)
    m = 1e-3
    return (lo > -10.0 + m) and (hi < -m) and w["alpha"] >= 0.0


def kernel(**inputs) -> np.ndarray:
    x = _np32(inputs["x"])
    assert x.shape == (B, T, N + 1)
    w = _fold_weights(inputs)
    ln_affine = not (np.all(w["g11"] == 1.0) and np.all(w["be11"] == 0.0)
                     and np.all(w["g21"] == 1.0) and np.all(w["be21"] == 0.0))
    general_tail = not _tail_is_degenerate(w)
    BPC = B // N_CORES
    built = _get_built(w, BPC, ln_affine, general_tail)

    # per-core t-major layout with a ones-row at t=64 (matmul bias trick)
    in_maps = []
    for c in range(N_CORES):
        xs = x[c * BPC:(c + 1) * BPC]          # [BPC, T, N+1]
        xp = np.empty((T + 1, BPC, N + 1), np.float32)
        xp[:T] = xs.transpose(1, 0, 2)
        xp[T] = 1.0
        in_maps.append({"x": xp, "d": np.ascontiguousarray(xp[:, :, N])})
    res = run_bass_kernel_spmd(built.nc, in_maps, list(range(N_CORES)))
    out = np.concatenate([r["y"] for r in res.results], axis=0)
    return out.astype(np.float32)


if __name__ == "__main__":
    rng = np.random.default_rng(0)
    print("kernel module ok")
